# revision 10
# baseline (speedup 1.0000x reference)
"""Trainium2 Bass kernel for nn_EquivariantBlock (EGNN block: 2 GCL layers +
equivariant coord update) on 8 NeuronCores.

Strategy: host sorts edges by row and shards them by destination-node range so
each core owns a contiguous block of 2560 nodes and all edges that aggregate
into them. Per layer, each core builds bf16 gather tables A=h@W1a (local rows)
and B=h@W1b (AllGathered across cores), dma_gathers per-edge columns in
feature-major layout, runs the edge MLP with PE matmuls (f32 PSUM accum),
and scatters edge features into per-window PSUM via one-hot matmuls (exact
f32 segment sums). The kernel program is JIT-built per input so sorted-edge
window structure is compile-time; all 8 cores run one SPMD program with
per-core data.
"""

import numpy as np

import concourse.bacc as bacc
import concourse.bass as bass
import concourse.mybir as mybir
import concourse.tile as tile
from concourse.bass_utils import run_bass_kernel_spmd

F32 = mybir.dt.float32
BF16 = mybir.dt.bfloat16
I16 = mybir.dt.int16

NCORES = 8
H = 128
WIN = 128          # nodes per scatter window
SC = 512           # edges per superchunk
CH = 128           # edges per chunk (PE partition dim)
NORM = 100.0
NORM_CONST = 1.0


# ---------------------------------------------------------------------------
# host-side preparation
# ---------------------------------------------------------------------------

def _as_np(v):
    return np.asarray(v)


def _host_prep(h, x, edge_index, edge_attr, node_mask, edge_mask, params):
    N = h.shape[0]
    E = edge_index.shape[1]
    npc = -(-N // (NCORES * WIN)) * WIN          # nodes per core (mult of 128)
    n_pad = npc * NCORES
    nwin = npc // WIN

    row = _as_np(edge_index[0]).astype(np.int64)
    col = _as_np(edge_index[1]).astype(np.int64)
    ea = _as_np(edge_attr).astype(np.float32)[:, 0]
    emask = _as_np(edge_mask).astype(np.float32)[:, 0]
    xf = _as_np(x).astype(np.float32)
    hf = _as_np(h).astype(np.float32)

    # coord2diff on host (inputs only, never updated before use)
    cd = xf[row] - xf[col]                       # [E,3]
    radial = np.sum(cd * cd, axis=1)             # [E]
    cdn = cd / (np.sqrt(radial + 1e-8)[:, None] + NORM_CONST)

    # sort edges by row, bucket into (core, window)
    order = np.argsort(row, kind="stable")
    row_s, col_s = row[order], col[order]
    ea_s, emask_s = ea[order], emask[order]
    radial_s, cdn_s = radial[order], cdn[order]

    win_of = row_s // WIN                        # global window id [0, nwin*8)
    # counts per global window
    counts = np.bincount(win_of, minlength=nwin * NCORES)
    w_e = int(max(SC, -(-counts.max() // SC) * SC))   # uniform window edge cap
    e_c = w_e * nwin                             # per-core padded edge count

    # per-(core,window) slot assignment
    starts = np.zeros(nwin * NCORES, dtype=np.int64)
    starts[1:] = np.cumsum(counts)[:-1]

    # destination position for each sorted edge
    within = np.arange(E, dtype=np.int64) - starts[win_of]
    core_of = win_of // nwin
    win_local = win_of % nwin
    dst = win_local * w_e + within               # position inside its core

    # per-core padded arrays
    rowl_c = np.zeros((NCORES, e_c), np.float32)       # row - window_base
    rowloc_c = np.zeros((NCORES, e_c), np.int64)       # row - core_base
    col_c = np.zeros((NCORES, e_c), np.int64)
    ea_c = np.zeros((NCORES, e_c), np.float32)
    emask_c = np.zeros((NCORES, e_c), np.float32)
    radial_c = np.zeros((NCORES, e_c), np.float32)
    cdn_c = np.zeros((NCORES, e_c, 3), np.float32)

    rowl_c[core_of, dst] = (row_s - win_of * WIN).astype(np.float32)
    rowloc_c[core_of, dst] = row_s - core_of * npc
    col_c[core_of, dst] = col_s
    ea_c[core_of, dst] = ea_s
    emask_c[core_of, dst] = emask_s
    radial_c[core_of, dst] = radial_s
    cdn_c[core_of, dst] = cdn_s

    # params -> layer list: (W1[258,128], b1, W2, b2, extra)
    gcl = params["gcl"]
    eq = params["equiv"]

    def g(p, k):
        return _as_np(p[k]).astype(np.float32)

    layers = []
    for lp in gcl:
        layers.append(dict(
            W1=g(lp, "eW1"), b1=g(lp, "eb1"), W2=g(lp, "eW2"), b2=g(lp, "eb2"),
            aW=g(lp, "aW"), ab=float(_as_np(lp["ab"])[0]),
            nW1=g(lp, "nW1"), nb1=g(lp, "nb1"), nW2=g(lp, "nW2"), nb2=g(lp, "nb2"),
            kind="gcl",
        ))
    layers.append(dict(
        W1=g(eq, "W1"), b1=g(eq, "b1"), W2=g(eq, "W2"), b2=g(eq, "b2"),
        W3=g(eq, "W3"), kind="equiv",
    ))

    # eaC_l = radial*W1c[0] + attr*W1c[1] + b1   (fm [128, e_c], bf16)
    ea_cons = []
    for lp in layers:
        W1c = lp["W1"][256:258, :]               # [2,128]
        for c in range(NCORES):
            v = (np.outer(W1c[0], radial_c[c]) + np.outer(W1c[1], ea_c[c])
                 + lp["b1"][:, None])
            ea_cons.append((c, v.astype(np.float32)))
    eaC = np.zeros((NCORES, 128, 3 * e_c), np.float32)
    for i, (c, v) in enumerate(ea_cons):
        l = i // NCORES
        eaC[c][:, l * e_c:(l + 1) * e_c] = v

    # gather index tiles: wrapped in 16 partitions, replicated to 128
    def wrap_idx(idx):                            # [e_c] -> [128, e_c//16] i16
        t = idx.reshape(e_c // 16, 16).T.astype(np.int16)   # [16, e_c/16]
        return np.tile(t, (8, 1))

    colidx_c = np.stack([wrap_idx(col_c[c]) for c in range(NCORES)])
    rowidx_c = np.stack([wrap_idx(rowloc_c[c]) for c in range(NCORES)])

    def em(v):                                    # [e_c] -> [128, e_c//128]
        return np.ascontiguousarray(v.reshape(e_c // CH, CH).T)

    rowl_em = np.stack([em(rowl_c[c]) for c in range(NCORES)])
    emask_em = np.stack([em(emask_c[c]) for c in range(NCORES)])
    # cdn em layout: [128, chunk*3 + k]
    cdn_em = np.stack([
        np.ascontiguousarray(
            cdn_c[c].reshape(e_c // CH, CH, 3).transpose(1, 0, 2).reshape(CH, -1))
        for c in range(NCORES)])

    # padded node-level tensors
    h_pad = np.zeros((n_pad, H), np.float32); h_pad[:N] = hf
    x_pad = np.zeros((n_pad, 3), np.float32); x_pad[:N] = xf
    nm_pad = np.zeros(n_pad, np.float32)
    nm_pad[:N] = _as_np(node_mask).astype(np.float32)[:, 0]

    hT = np.ascontiguousarray(h_pad.T)            # [128, n_pad]
    xT = np.ascontiguousarray(x_pad.T)            # [3, n_pad]

    cfg = dict(N=N, E=E, npc=npc, n_pad=n_pad, nwin=nwin, w_e=w_e, e_c=e_c,
               layers=layers)

    per_core = []
    for c in range(NCORES):
        sl = slice(c * npc, (c + 1) * npc)
        per_core.append(dict(
            h_fm=np.ascontiguousarray(hT[:, sl]),
            x_fm=np.ascontiguousarray(xT[:, sl]),
            nmask_bc=np.ascontiguousarray(
                np.broadcast_to(nm_pad[sl], (128, npc))).astype(ml_bf16),
            colidx=colidx_c[c],
            rowidx=rowidx_c[c],
            rowl_em=rowl_em[c].astype(np.float32),
            emask_em=emask_em[c].astype(np.float32),
            cdn_em=cdn_em[c].astype(ml_bf16),
            eaC=eaC[c].astype(ml_bf16),
        ))
    return cfg, per_core


try:
    import ml_dtypes
    ml_bf16 = ml_dtypes.bfloat16
except ImportError:  # pragma: no cover
    ml_bf16 = np.float32


# ---------------------------------------------------------------------------
# const packing
# ---------------------------------------------------------------------------

class _Pack:
    def __init__(self, dtype):
        self.cols = 0
        self.slots = {}
        self.chunks = []
        self.dtype = dtype

    def add(self, name, arr):                     # arr [p, w]
        p, w = arr.shape
        self.slots[name] = (self.cols, w, p)
        self.chunks.append((self.cols, arr))
        self.cols += w

    def build(self):
        buf = np.zeros((128, self.cols), self.dtype)
        for off, arr in self.chunks:
            buf[: arr.shape[0], off:off + arr.shape[1]] = arr
        return buf


def _pack_consts(cfg):
    pf = _Pack(np.float32)
    pb = _Pack(ml_bf16)
    for li, lp in enumerate(cfg["layers"]):
        pf.add(f"W1a{li}", lp["W1"][0:128, :])
        pf.add(f"W1b{li}", lp["W1"][128:256, :])
        pb.add(f"W2{li}", lp["W2"].astype(ml_bf16))
        b2row4 = np.tile(lp["b2"], 4)[None, :]            # [1,512]
        pb.add(f"b2row{li}", b2row4.astype(ml_bf16))
        if lp["kind"] == "gcl":
            aWbc = np.broadcast_to(lp["aW"][:, 0], (128, 128))
            pb.add(f"aWbc{li}", np.tile(aWbc, (1, 4)).astype(ml_bf16))
            pf.add(f"nW1a{li}", lp["nW1"][0:128, :])
            pf.add(f"nW1b{li}", lp["nW1"][128:256, :] / NORM)
            pf.add(f"nW2{li}", lp["nW2"])
            pf.add(f"nb1{li}", lp["nb1"][:, None])
            pf.add(f"nb2{li}", lp["nb2"][:, None])
        else:
            W3bc = np.broadcast_to(lp["W3"][:, 0] / NORM, (128, 128))
            pb.add(f"W3bc{li}", np.tile(W3bc, (1, 4)).astype(ml_bf16))
    pb.add("iota", np.broadcast_to(np.arange(128, dtype=np.float32),
                                   (128, 128)).astype(ml_bf16))
    pb.add("ones", np.ones((1, 128), ml_bf16))
    return pf, pb


# ---------------------------------------------------------------------------
# device program
# ---------------------------------------------------------------------------

def _build(cfg, cf_np, cb_np):
    npc, nwin, w_e, e_c = cfg["npc"], cfg["nwin"], cfg["w_e"], cfg["e_c"]
    n_pad = cfg["n_pad"]
    scw = w_e // SC                      # superchunks per window
    layers = cfg["layers"]

    nc = bacc.Bacc("TRN2", target_bir_lowering=False, debug=False,
                   num_devices=NCORES)

    # I/O
    t_h = nc.dram_tensor("h_fm", [128, npc], F32, kind="ExternalInput")
    t_x = nc.dram_tensor("x_fm", [3, npc], F32, kind="ExternalInput")
    t_nm = nc.dram_tensor("nmask_bc", [128, npc], BF16, kind="ExternalInput")
    t_ci = nc.dram_tensor("colidx", [128, e_c // 16], I16, kind="ExternalInput")
    t_ri = nc.dram_tensor("rowidx", [128, e_c // 16], I16, kind="ExternalInput")
    t_rl = nc.dram_tensor("rowl_em", [128, e_c // CH], F32, kind="ExternalInput")
    t_em = nc.dram_tensor("emask_em", [128, e_c // CH], F32, kind="ExternalInput")
    t_cd = nc.dram_tensor("cdn_em", [128, 3 * e_c // CH], BF16, kind="ExternalInput")
    t_ea = nc.dram_tensor("eaC", [128, 3 * e_c], BF16, kind="ExternalInput")
    t_cf = nc.dram_tensor("cf", list(cf_np.shape), F32, kind="ExternalInput")
    t_cb = nc.dram_tensor("cb", list(cb_np.shape), BF16, kind="ExternalInput")
    t_ho = nc.dram_tensor("h_out", [128, npc], F32, kind="ExternalOutput")
    t_xo = nc.dram_tensor("x_out", [3, npc], F32, kind="ExternalOutput")

    # internal DRAM
    d_A = nc.dram_tensor("A_tab", [npc, 128], BF16)
    d_Bown = nc.dram_tensor("B_own", [npc, 128], BF16)
    d_Bfull = nc.dram_tensor("B_full", [n_pad, 128], BF16, addr_space="Shared")

    cf_slot, cb_slot = cfg["cf_slots"], cfg["cb_slots"]

    with tile.TileContext(nc) as tc:
        with (
            tc.tile_pool(name="persist", bufs=1) as pp,
            tc.tile_pool(name="gath", bufs=3) as pg,
            tc.tile_pool(name="work", bufs=3) as pw,
            tc.tile_pool(name="small", bufs=3) as ps,
            tc.tile_pool(name="stage", bufs=2) as pst,
            tc.tile_pool(name="px2", bufs=2, space="PSUM") as px2,
            tc.tile_pool(name="pagg", bufs=2, space="PSUM") as pagg,
            tc.tile_pool(name="pnm", bufs=2, space="PSUM") as pnm,
        ):
            # ---- persistent loads ----
            s_h = pp.tile([128, npc], F32, tag="h")
            s_x = pp.tile([3, npc], F32, tag="x")
            s_nm = pp.tile([128, npc], BF16, tag="nm")
            s_agg = pp.tile([128, npc], F32, tag="agg")
            s_aggx = pp.tile([3, npc], F32, tag="aggx")
            s_ci = pp.tile([128, e_c // 16], I16, tag="ci")
            s_ri = pp.tile([128, e_c // 16], I16, tag="ri")
            s_rl = pp.tile([128, e_c // CH], F32, tag="rl")
            s_em = pp.tile([128, e_c // CH], F32, tag="em")
            s_cd = pp.tile([128, 3 * e_c // CH], BF16, tag="cd")
            s_cf = pp.tile([128, cf_np.shape[1]], F32, tag="cf")
            s_cb = pp.tile([128, cb_np.shape[1]], BF16, tag="cb")

            nc.sync.dma_start(s_h[:], t_h.ap())
            nc.sync.dma_start(s_x[:], t_x.ap())
            nc.sync.dma_start(s_nm[:], t_nm.ap())
            nc.sync.dma_start(s_ci[:], t_ci.ap())
            nc.sync.dma_start(s_ri[:], t_ri.ap())
            nc.sync.dma_start(s_rl[:], t_rl.ap())
            nc.sync.dma_start(s_em[:], t_em.ap())
            nc.sync.dma_start(s_cd[:], t_cd.ap())
            nc.sync.dma_start(s_cf[:], t_cf.ap())
            nc.sync.dma_start(s_cb[:], t_cb.ap())

            def cf(name):
                off, w, p = cf_slot[name]
                return s_cf[:p, off:off + w]

            def cb(name):
                off, w, p = cb_slot[name]
                return s_cb[:p, off:off + w]

            AluOp = mybir.AluOpType
            Act = mybir.ActivationFunctionType

            # ---------------- per-layer ----------------
            for li, lp in enumerate(layers):
                is_gcl = lp["kind"] == "gcl"

                # ---- build A (local) and B (allgather) tables ----
                stg_a = pst.tile([128, nwin * 128], BF16, tag="stga")
                stg_b = pst.tile([128, nwin * 128], BF16, tag="stgb")
                for w in range(nwin):
                    pt = pnm.tile([128, 128], F32, tag="tbl")
                    hwin = s_h[:, w * 128:(w + 1) * 128]
                    nc.tensor.matmul(pt[:], hwin, cf(f"W1a{li}"),
                                     start=True, stop=True)
                    nc.scalar.copy(stg_a[:, w * 128:(w + 1) * 128], pt[:])
                    pt2 = pnm.tile([128, 128], F32, tag="tbl")
                    nc.tensor.matmul(pt2[:], hwin, cf(f"W1b{li}"),
                                     start=True, stop=True)
                    nc.scalar.copy(stg_b[:, w * 128:(w + 1) * 128], pt2[:])
                # SBUF [128n, win, 128f] -> DRAM node-major [npc, 128]
                a_view = d_A.ap().rearrange("(w n) f -> n w f", w=nwin)
                b_view = d_Bown.ap().rearrange("(w n) f -> n w f", w=nwin)
                nc.sync.dma_start(a_view, stg_a[:].rearrange(
                    "n (w f) -> n w f", w=nwin))
                nc.sync.dma_start(b_view, stg_b[:].rearrange(
                    "n (w f) -> n w f", w=nwin))
                nc.gpsimd.collective_compute(
                    "AllGather", AluOp.bypass,
                    replica_groups=[list(range(NCORES))],
                    ins=[d_Bown.ap()], outs=[d_Bfull.ap()],
                )

                # ---- edge phase ----
                for w in range(nwin):
                    p_ag = pagg.tile([128, 128], F32, tag="aggw")
                    for s in range(scw):
                        sc_i = w * scw + s           # superchunk index
                        e0 = sc_i * SC
                        g_a = pg.tile([128, 1, SC], BF16, tag="ga")
                        g_b = pg.tile([128, 1, SC], BF16, tag="gb")
                        nc.gpsimd.dma_gather(
                            g_a[:], d_A.ap(), s_ri[:, e0 // 16:(e0 + SC) // 16],
                            SC, SC, 128, transpose=True)
                        nc.gpsimd.dma_gather(
                            g_b[:], d_Bfull.ap(), s_ci[:, e0 // 16:(e0 + SC) // 16],
                            SC, SC, 128, transpose=True)
                        w_ea = pw.tile([128, SC], BF16, tag="ea")
                        nc.sync.dma_start(
                            w_ea[:], t_ea.ap()[:, li * e_c + e0:li * e_c + e0 + SC])
                        # X1 = A + B + eaC ; m1 = silu(X1)
                        x1 = pw.tile([128, SC], BF16, tag="x1")
                        nc.vector.tensor_tensor(
                            x1[:], g_a[:, 0, :], g_b[:, 0, :], AluOp.add)
                        nc.vector.tensor_tensor(x1[:], x1[:], w_ea[:], AluOp.add)
                        m1 = pw.tile([128, SC], BF16, tag="m1")
                        nc.scalar.activation(m1[:], x1[:], Act.Silu)
                        # X2 em (4 chunks share psum): b2 + m1T_c @ W2
                        p_x2 = px2.tile([128, SC], F32, tag="x2")
                        nc.tensor.matmul(p_x2[:], cb("ones"),
                                         cb(f"b2row{li}"), start=True, stop=False)
                        for ci4 in range(SC // CH):
                            nc.tensor.matmul(
                                p_x2[:, ci4 * CH:(ci4 + 1) * CH],
                                m1[:, ci4 * CH:(ci4 + 1) * CH],
                                cb(f"W2{li}"),
                                start=False, stop=(ci4 == SC // CH - 1))
                        m2 = pw.tile([128, SC], BF16, tag="m2")
                        nc.scalar.activation(m2[:], p_x2[:], Act.Silu)
                        # att / cm  -> scalar per edge (em [128, 4])
                        attp = ps.tile([128, SC // CH], F32, tag="attp")
                        bcname = f"aWbc{li}" if is_gcl else f"W3bc{li}"
                        scr = pw.tile([128, SC], BF16, tag="scr")
                        nc.vector.tensor_tensor(scr[:], m2[:], cb(bcname),
                                                AluOp.mult)
                        nc.vector.tensor_reduce(
                            attp[:], scr[:].rearrange("p (c f) -> p c f", f=CH),
                            mybir.AxisListType.X, AluOp.add)
                        attm = ps.tile([128, SC // CH], F32, tag="attm")
                        if is_gcl:
                            nc.scalar.activation(attm[:], attp[:], Act.Sigmoid,
                                                 bias=lp["ab"])
                            nc.vector.tensor_tensor(
                                attm[:], attm[:],
                                s_em[:, sc_i * 4:sc_i * 4 + SC // CH], AluOp.mult)
                        else:
                            nc.vector.tensor_tensor(
                                attm[:], attp[:],
                                s_em[:, sc_i * 4:sc_i * 4 + SC // CH], AluOp.mult)
                        # scatter: onehot' = (iota == rowl) * attm ; agg += ...
                        for ci4 in range(SC // CH):
                            chn = sc_i * 4 + ci4
                            oh = pw.tile([128, CH], BF16, tag="oh")
                            nc.vector.tensor_scalar(
                                oh[:], cb("iota"),
                                s_rl[:, chn:chn + 1], attm[:, ci4:ci4 + 1],
                                AluOp.is_equal, AluOp.mult)
                            lhs = (m2[:, ci4 * CH:(ci4 + 1) * CH] if is_gcl
                                   else s_cd[:, chn * 3:chn * 3 + 3])
                            nc.tensor.matmul(
                                p_ag[: (128 if is_gcl else 3), :], lhs, oh[:],
                                start=(s == 0 and ci4 == 0),
                                stop=(s == scw - 1 and ci4 == SC // CH - 1))
                    # evict window agg
                    dst = s_agg if is_gcl else s_aggx
                    rows = 128 if is_gcl else 3
                    nc.scalar.copy(dst[:rows, w * 128:(w + 1) * 128],
                                   p_ag[:rows, :])

                # ---- node phase ----
                if is_gcl:
                    nss = min(512, npc)
                    for ns in range(npc // nss):
                        nsl = slice(ns * nss, (ns + 1) * nss)
                        pu = pnm.tile([128, nss], F32, tag="node")
                        nc.tensor.matmul(pu[:], cf(f"nW1a{li}"), s_h[:, nsl],
                                         start=True, stop=False)
                        nc.tensor.matmul(pu[:], cf(f"nW1b{li}"), s_agg[:, nsl],
                                         start=False, stop=True)
                        u = pw.tile([128, nss], F32, tag="u")
                        nc.scalar.activation(u[:], pu[:], Act.Silu,
                                             bias=cf(f"nb1{li}"))
                        pv = pnm.tile([128, nss], F32, tag="node")
                        nc.tensor.matmul(pv[:], cf(f"nW2{li}"), u[:],
                                         start=True, stop=True)
                        v = pw.tile([128, nss], F32, tag="v")
                        nc.scalar.activation(v[:], pv[:], Act.Identity,
                                             bias=cf(f"nb2{li}"))
                        nc.vector.tensor_tensor(s_h[:, nsl], s_h[:, nsl], v[:],
                                                AluOp.add)
                        nc.vector.tensor_tensor(s_h[:, nsl], s_h[:, nsl],
                                                s_nm[:, nsl], AluOp.mult)
                else:
                    # x = (x + aggx) * mask   (aggx already scaled by W3/100)
                    nc.vector.tensor_tensor(s_x[:], s_x[:], s_aggx[:], AluOp.add)
                    nc.vector.tensor_tensor(s_x[:], s_x[:], s_nm[:3, :],
                                            AluOp.mult)

            # final h mask + outputs
            nc.vector.tensor_tensor(s_h[:], s_h[:], s_nm[:], AluOp.mult)
            nc.sync.dma_start(t_ho.ap(), s_h[:])
            nc.sync.dma_start(t_xo.ap(), s_x[:])

    nc.compile()
    return nc


# ---------------------------------------------------------------------------
# entry point
# ---------------------------------------------------------------------------

def _run(inputs, sim=False):
    cfg, per_core = _host_prep(**inputs)
    pf, pb = _pack_consts(cfg)
    cf_np, cb_np = pf.build(), pb.build()
    cfg["cf_slots"], cfg["cb_slots"] = pf.slots, pb.slots
    nc = _build(cfg, cf_np, cb_np)

    in_maps = []
    for c in range(NCORES):
        d = dict(per_core[c])
        m = {
            "h_fm": d["h_fm"], "x_fm": d["x_fm"],
            "nmask_bc": np.asarray(d["nmask_bc"]),
            "colidx": d["colidx"], "rowidx": d["rowidx"],
            "rowl_em": np.asarray(d["rowl_em"]),
            "emask_em": np.asarray(d["emask_em"]),
            "cdn_em": np.asarray(d["cdn_em"]),
            "eaC": np.asarray(d["eaC"]),
            "cf": cf_np, "cb": np.asarray(cb_np),
        }
        in_maps.append(m)

    if sim:
        from concourse.bass_interp import MultiCoreSim
        msim = MultiCoreSim(nc, NCORES)
        for c in range(NCORES):
            for k, v in in_maps[c].items():
                msim.cores[c].tensor(k)[:] = v
        msim.simulate()
        results = [{"h_out": np.array(msim.cores[c].mem_tensor("h_out")),
                    "x_out": np.array(msim.cores[c].mem_tensor("x_out"))}
                   for c in range(NCORES)]
    else:
        res = run_bass_kernel_spmd(nc, in_maps, core_ids=list(range(NCORES)))
        results = res.results

    N = cfg["N"]
    hT = np.concatenate([results[c]["h_out"] for c in range(NCORES)], axis=1)
    xT = np.concatenate([results[c]["x_out"] for c in range(NCORES)], axis=1)
    return np.ascontiguousarray(hT.T[:N]), np.ascontiguousarray(xT.T[:N])


def kernel(**inputs):
    return _run(inputs, sim=False)
